# revision 14
# baseline (speedup 1.0000x reference)
"""Causal self-attention (non-masked softmax path) for TRN2, 8 NeuronCores.

Sharding: 2-way data parallel over batch x 4-way tensor parallel over heads.
Core c handles batch b = c // 4, head group g = c % 4 (heads 4g..4g+3).
Each core computes its QKV projection slice, full attention for its 4 heads,
and the row-parallel c_proj partial; the host sums the 4 partials per batch
(the all-reduce of row-parallel tensor parallelism) and adds b_proj.

All matmuls take bf16 inputs (fp32 matmul on TRN2 runs as 2 passes at 2
bytes/cycle = 4x slower) and accumulate in fp32 PSUM. Measured end-to-end
error vs the fp32 reference is ~6e-3 scale-relative.

Per-core dataflow:
  xT [1024, 2048] bf16 (host pre-transposed; matmuls contract over partitions)
  qkT = Wqk.T @ x.T [512, 2048] bf16, q pre-scaled by 1/sqrt(hd) via W on host
  v   = x @ Wv [2048, 4 x (64|1)] bf16 -- augmented with a ones column per
        head so the PV matmul (lhsT [128, 65]) also produces the softmax
        denominator in psum row 64.
  S^T chunks [128 k, 512 q] per head, K=64 row-packed 2 heads per PE pass
  (row-group packing runs concurrently; col-group packing does not).
  exp on ACT over pair-fused [128, 1024] psum -> bf16 E tiles.
  Normalize y^T after PV: stage pvd to SBUF (frees psum banks fast), batched
  reciprocal of the 4 denominator rows, selector-matmul broadcast, DVE mul.
  c_proj contracts the 256 group features; partial out stays fp32.

A single PSUM pool with tags "s" (2 x 2 banks) and "pvd" (4 x 1 bank) is
shared by all phases so projection, attention, and c_proj pipeline into
each other instead of serializing on pool-scope boundaries. PSUM->SBUF
copies that would collide with DVE work run on the otherwise-idle ACT
engine (activation Copy/Identity).
"""

import numpy as np

B, T, H, NH, HD = 2, 2048, 1024, 16, 64
P = 128
FG = 256          # features per head group (4 heads x 64)
VC = 65           # v columns per head incl. the ones column
NQ = 512          # Tq chunk (psum free dim)
NJ = T // NQ      # 4
NI = T // P       # 16 key chunks
KH = H // P       # 8 hidden chunks
NCORES = 8

_CACHE = {}


def _build():
    import concourse.bacc as bacc
    import concourse.mybir as mybir
    import concourse.tile as tile

    fp32 = mybir.dt.float32
    bf16 = mybir.dt.bfloat16

    nc = bacc.Bacc("TRN2", debug=False)
    xT = nc.dram_tensor("xT", [H, T], bf16, kind="ExternalInput").ap()
    wqkv = nc.dram_tensor("wqkv", [H, 3 * FG], bf16, kind="ExternalInput").ap()
    bqk = nc.dram_tensor("bqk", [2 * FG], fp32, kind="ExternalInput").ap()
    bv = nc.dram_tensor("bv", [FG], bf16, kind="ExternalInput").ap()
    wp = nc.dram_tensor("wp", [FG, H], bf16, kind="ExternalInput").ap()
    out = nc.dram_tensor("out", [T, H], fp32, kind="ExternalOutput").ap()

    with tile.TileContext(nc) as tc:
        _emit(nc, tc, mybir, xT, wqkv, bqk, bv, wp, out)
    nc.compile()
    return nc


def _emit(nc, tc, mybir, xT, wqkv, bqk, bv, wp, out):
    from contextlib import ExitStack

    fp32 = mybir.dt.float32
    bf16 = mybir.dt.bfloat16
    Exp = mybir.ActivationFunctionType.Exp
    Ident = mybir.ActivationFunctionType.Identity
    Copy = mybir.ActivationFunctionType.Copy

    W3 = 3 * FG  # 768, wqkv row width

    with ExitStack() as ctx:
        pool = lambda name, bufs=1, space="SBUF": ctx.enter_context(
            tc.tile_pool(name=name, bufs=bufs, space=space)
        )

        const = pool("const")
        ones = const.tile([1, P], bf16)
        nc.vector.memset(ones[:], 1.0)
        bias0 = const.tile([P, 1], fp32)
        nc.vector.memset(bias0[:], 0.0)
        # selector weights for the denominator broadcast: sel[k, 64h+m]
        # = 1 iff k == 32h; sel_h.T @ rcp replicates head h's denominator
        # row (32h of rcp) across 64 output partitions starting at base 0.
        sel = const.tile([P, 4 * 64], fp32)
        nc.vector.memset(sel[:], 0.0)
        for h in range(4):
            nc.vector.memset(sel[32 * h : 32 * h + 1, h * 64 : (h + 1) * 64], 1.0)
        bqk_sb = const.tile([P, 4], fp32)
        nc.sync.dma_start(bqk_sb[:], bqk.rearrange("(m p) -> p m", p=P))
        bv_sb = const.tile([1, FG], bf16)
        nc.sync.dma_start(bv_sb[:], bv.rearrange("(o f) -> o f", o=1))

        xt_sb = pool("xt").tile([P, KH * T], bf16)
        w_sb = pool("w").tile([P, KH * W3], bf16)
        wp_sb = pool("wp").tile([P, 2 * H], bf16)
        for k in range(KH):
            nc.sync.dma_start(xt_sb[:, k * T : (k + 1) * T], xT[k * P : (k + 1) * P, :])
            nc.sync.dma_start(
                w_sb[:, k * W3 : (k + 1) * W3], wqkv[k * P : (k + 1) * P, :]
            )
        for kk in range(2):
            nc.sync.dma_start(
                wp_sb[:, kk * H : (kk + 1) * H], wp[kk * P : (kk + 1) * P, :]
            )

        qk_sb = pool("qk").tile([P, 4 * T], bf16)   # feat chunks: q0 q1 k0 k1
        v_sb = pool("v").tile([P, NI * 4 * VC], bf16)
        v4 = v_sb.rearrange("p (t h c) -> p t h c", t=NI, h=4, c=VC)
        nc.vector.memset(v4[:, :, :, 64:65], 1.0)   # denominator ones columns
        y_sb = pool("y").tile([P, 2 * T], bf16)     # y^T, feat pair chunks x T

        # one psum pool for all phases: tag "s" = 2 slots x 2 banks,
        # tag "pvd" = 4 slots x 1 bank (16KB/partition total)
        ps_pool = ctx.enter_context(tc.tile_pool(name="ps", bufs=2, space="PSUM"))
        epool = pool("e", bufs=3)
        nrm = pool("nrm", bufs=2)
        pvs_pool = pool("pvs", bufs=4)
        outp = pool("outp", bufs=3)

        # ---- Phase A: projections (pair-0 k/q first so attention starts early)
        for m in (2, 0, 3, 1):
            for j in range(NJ):
                ps = ps_pool.tile([P, 2 * NQ], fp32, tag="s", name=f"qk{m}_{j}")
                for k in range(KH):
                    nc.tensor.matmul(
                        ps[:, 0:NQ],
                        w_sb[:, k * W3 + m * P : k * W3 + (m + 1) * P],
                        xt_sb[:, k * T + j * NQ : k * T + (j + 1) * NQ],
                        start=(k == 0),
                        stop=(k == KH - 1),
                    )
                nc.scalar.activation(  # copy+bias on ACT (idle during phase A)
                    qk_sb[:, m * T + j * NQ : m * T + (j + 1) * NQ],
                    ps[:, 0:NQ],
                    Ident,
                    bias=bqk_sb[:, m : m + 1],
                )
        for t in range(NI):  # v in natural layout, strided into [64|1] slots
            ps = ps_pool.tile([P, NQ], fp32, tag="pvd", bufs=4, name=f"v{t}")
            for k in range(KH):
                nc.tensor.matmul(
                    ps[:, 0:FG],
                    xt_sb[:, k * T + t * P : k * T + (t + 1) * P],
                    w_sb[:, k * W3 + 2 * FG : (k + 1) * W3],
                    start=(k == 0),
                    stop=False,
                )
            nc.tensor.matmul(  # += ones.T @ bv  (bias broadcast over rows)
                ps[:, 0:FG],
                ones[0:1, :],
                bv_sb[0:1, :],
                start=False,
                stop=True,
            )
            nc.vector.tensor_copy(
                v4[:, t, :, 0:64],
                ps[:, 0:FG].rearrange("p (h c) -> p h c", h=4, c=64),
            )

        # ---- Phase B: attention ----
        for j in range(NJ):
            pvd = [
                ps_pool.tile([VC, NQ], fp32, tag="pvd", bufs=4, name=f"pvd{j}_{h}")
                for h in range(4)
            ]
            for i in range(NI):
                for p in range(2):  # head pairs (2p, 2p+1)
                    sp = ps_pool.tile([P, 2 * NQ], fp32, tag="s", name=f"s{j}_{i}_{p}")
                    for hh in range(2):
                        bp = 64 * hh
                        # S^T chunk: K=64, row-packed -> runs concurrently
                        nc.tensor.matmul(
                            sp[:, hh * NQ : (hh + 1) * NQ],
                            qk_sb[
                                bp : bp + 64,
                                (2 + p) * T + i * P : (2 + p) * T + (i + 1) * P,
                            ],
                            qk_sb[
                                bp : bp + 64, p * T + j * NQ : p * T + (j + 1) * NQ
                            ],
                            start=True,
                            stop=True,
                            tile_position=(bp, 0),
                        )
                    e = epool.tile([P, 2 * NQ], bf16, tag="e")
                    nc.scalar.activation(e[:], sp[:], Exp, bias=bias0[:, 0:1])
                    for hh in range(2):
                        h = 2 * p + hh
                        nc.tensor.matmul(  # y^T rows 0..63, denominator row 64
                            pvd[h][:],
                            v4[:, i, h, :],
                            e[:, hh * NQ : (hh + 1) * NQ],
                            start=(i == 0),
                            stop=(i == NI - 1),
                        )
            # normalize: stage to SBUF (frees pvd banks), batched reciprocal,
            # selector broadcast, all-SBUF multiplies
            pvs = [
                pvs_pool.tile([VC, NQ], fp32, tag="pvs", name=f"pvs{j}_{h}")
                for h in range(4)
            ]
            for h in range(4):
                nc.vector.tensor_copy(pvs[h][:], pvd[h][:])
            dg = nrm.tile([P, NQ], fp32, tag="dg")
            nc.vector.memset(dg[:], 1.0)  # keep untouched rows finite
            for h in range(4):
                nc.vector.tensor_copy(dg[32 * h : 32 * h + 1, :], pvs[h][64:65, :])
            rcp = nrm.tile([P, NQ], fp32, tag="rcp")
            nc.vector.reciprocal(rcp[:], dg[:])
            for h in range(4):
                p, hh = divmod(h, 2)
                bc = ps_pool.tile([64, NQ], fp32, tag="pvd", bufs=4, name=f"bc{j}_{h}")
                nc.tensor.matmul(
                    bc[:], sel[:, h * 64 : (h + 1) * 64], rcp[:], start=True, stop=True
                )
                bc_sb = nrm.tile([64, NQ], fp32, tag="bcs")
                nc.scalar.activation(bc_sb[:], bc[:], Copy)
                nc.vector.tensor_mul(  # both SB inputs at base partition 0
                    y_sb[
                        64 * hh : 64 * hh + 64,
                        p * T + j * NQ : p * T + (j + 1) * NQ,
                    ],
                    pvs[h][0:64, :],
                    bc_sb[:],
                )

        # ---- Phase C: c_proj partial (row-parallel) ----
        for mq in range(NI):
            for n in range(2):
                ps = ps_pool.tile([P, NQ], fp32, tag="pvd", bufs=4, name=f"c{mq}_{n}")
                for kk in range(2):
                    nc.tensor.matmul(
                        ps[:],
                        y_sb[:, kk * T + mq * P : kk * T + (mq + 1) * P],
                        wp_sb[:, kk * H + n * NQ : kk * H + (n + 1) * NQ],
                        start=(kk == 0),
                        stop=(kk == 1),
                    )
                ot = outp.tile([P, NQ], fp32, tag="o")
                nc.scalar.activation(ot[:], ps[:], Copy)  # ACT idle in phase C
                nc.sync.dma_start(
                    out[mq * P : (mq + 1) * P, n * NQ : (n + 1) * NQ], ot[:]
                )


def _get_nc():
    if "nc" not in _CACHE:
        _CACHE["nc"] = _build()
    return _CACHE["nc"]


def _make_in_maps(x, W_attn, b_attn, W_proj):
    import ml_dtypes

    bf = ml_dtypes.bfloat16
    x = np.asarray(x, np.float32)
    W_attn = np.asarray(W_attn, np.float32)
    b_attn = np.asarray(b_attn, np.float32)
    W_proj = np.asarray(W_proj, np.float32)
    scale = 1.0 / np.sqrt(np.float32(HD))
    in_maps = []
    for c in range(NCORES):
        b, g = divmod(c, 4)
        sl = slice(FG * g, FG * (g + 1))
        wq = W_attn[:, sl] * scale
        wk = W_attn[:, H:][:, sl]
        wv = W_attn[:, 2 * H :][:, sl]
        in_maps.append(
            {
                "xT": np.ascontiguousarray(x[b].T).astype(bf),
                "wqkv": np.ascontiguousarray(
                    np.concatenate([wq, wk, wv], axis=1)
                ).astype(bf),
                "bqk": np.concatenate(
                    [b_attn[sl] * scale, b_attn[H:][sl]]
                ).astype(np.float32),
                "bv": np.ascontiguousarray(b_attn[2 * H :][sl]).astype(bf),
                "wp": np.ascontiguousarray(W_proj[sl, :]).astype(bf),
            }
        )
    return in_maps


def _gather(results, b_proj):
    b_proj = np.asarray(b_proj, np.float32)
    y = np.empty((B, T, H), np.float32)
    for b in range(B):
        acc = results[4 * b]["out"].astype(np.float32)
        for g in range(1, 4):
            acc = acc + results[4 * b + g]["out"]
        y[b] = acc + b_proj[None, :]
    return y


def run(x, W_attn, b_attn, W_proj, b_proj, trace=False):
    from concourse.bass_utils import run_bass_kernel_spmd

    nc = _get_nc()
    in_maps = _make_in_maps(x, W_attn, b_attn, W_proj)
    res = run_bass_kernel_spmd(nc, in_maps, list(range(NCORES)), trace=trace)
    return _gather(res.results, b_proj), res


def kernel(x, W_attn, b_attn, W_proj, b_proj):
    y, _ = run(x, W_attn, b_attn, W_proj, b_proj, trace=False)
    return y


# revision 15
# speedup vs baseline: 1.0146x; 1.0146x over previous
"""Causal self-attention (non-masked softmax path) for TRN2, 8 NeuronCores.

Sharding: 2-way data parallel over batch x 4-way tensor parallel over heads.
Core c handles batch b = c // 4, head group g = c % 4 (heads 4g..4g+3).
Each core computes its QKV projection slice, full attention for its 4 heads,
and the row-parallel c_proj partial; the host sums the 4 partials per batch
(the all-reduce of row-parallel tensor parallelism) and adds b_proj.

All matmuls take bf16 inputs (fp32 matmul on TRN2 runs as 2 passes at 2
bytes/cycle = 4x slower) and accumulate in fp32 PSUM. Measured end-to-end
error vs the fp32 reference is ~6e-3 scale-relative.

Per-core dataflow:
  xT [1024, 2048] bf16 (host pre-transposed; matmuls contract over partitions)
  qkT = Wqk.T @ x.T [512, 2048] bf16, q pre-scaled by 1/sqrt(hd) via W on host
  v   = x @ Wv [2048, 4 x (64|1)] bf16 -- augmented with a ones column per
        head so the PV matmul (lhsT [128, 65]) also produces the softmax
        denominator in psum row 64.
  S^T chunks [128 k, 512 q] per head, K=64 row-packed 2 heads per PE pass
  (row-group packing runs concurrently; col-group packing does not).
  exp on ACT over pair-fused [128, 1024] psum -> bf16 E tiles.
  Normalize y^T after PV: stage pvd to SBUF (frees psum banks fast), batched
  reciprocal of the 4 denominator rows, selector-matmul broadcast, DVE mul.
  c_proj contracts the 256 group features; partial out stays fp32.

A single PSUM pool with tags "s" (2 x 2 banks) and "pvd" (4 x 1 bank) is
shared by all phases so projection, attention, and c_proj pipeline into
each other instead of serializing on pool-scope boundaries. PSUM->SBUF
copies that would collide with DVE work run on the otherwise-idle ACT
engine (activation Copy/Identity).
"""

import numpy as np

B, T, H, NH, HD = 2, 2048, 1024, 16, 64
P = 128
FG = 256          # features per head group (4 heads x 64)
VC = 65           # v columns per head incl. the ones column
NQ = 512          # Tq chunk (psum free dim)
NJ = T // NQ      # 4
NI = T // P       # 16 key chunks
KH = H // P       # 8 hidden chunks
NCORES = 8

_CACHE = {}


def _build():
    import concourse.bacc as bacc
    import concourse.mybir as mybir
    import concourse.tile as tile

    fp32 = mybir.dt.float32
    bf16 = mybir.dt.bfloat16

    nc = bacc.Bacc("TRN2", debug=False)
    xT = nc.dram_tensor("xT", [H, T], bf16, kind="ExternalInput").ap()
    wqkv = nc.dram_tensor("wqkv", [H, 3 * FG], bf16, kind="ExternalInput").ap()
    bqk = nc.dram_tensor("bqk", [2 * FG], fp32, kind="ExternalInput").ap()
    bv = nc.dram_tensor("bv", [FG], bf16, kind="ExternalInput").ap()
    wp = nc.dram_tensor("wp", [FG, H], bf16, kind="ExternalInput").ap()
    out = nc.dram_tensor("out", [T, H], fp32, kind="ExternalOutput").ap()

    with tile.TileContext(nc) as tc:
        _emit(nc, tc, mybir, xT, wqkv, bqk, bv, wp, out)
    nc.compile()
    return nc


def _emit(nc, tc, mybir, xT, wqkv, bqk, bv, wp, out):
    from contextlib import ExitStack

    fp32 = mybir.dt.float32
    bf16 = mybir.dt.bfloat16
    Exp = mybir.ActivationFunctionType.Exp
    Ident = mybir.ActivationFunctionType.Identity
    Copy = mybir.ActivationFunctionType.Copy

    W3 = 3 * FG  # 768, wqkv row width

    with ExitStack() as ctx:
        pool = lambda name, bufs=1, space="SBUF": ctx.enter_context(
            tc.tile_pool(name=name, bufs=bufs, space=space)
        )

        const = pool("const")
        ones = const.tile([1, P], bf16)
        nc.vector.memset(ones[:], 1.0)
        bias0 = const.tile([P, 1], fp32)
        nc.vector.memset(bias0[:], 0.0)
        # selector weights for the denominator broadcast: sel[k, 64h+m]
        # = 1 iff k == 32h; sel_h.T @ rcp replicates head h's denominator
        # row (32h of rcp) across 64 output partitions starting at base 0.
        sel = const.tile([P, 4 * 64], fp32)
        nc.vector.memset(sel[:], 0.0)
        for h in range(4):
            nc.vector.memset(sel[32 * h : 32 * h + 1, h * 64 : (h + 1) * 64], 1.0)
        bqk_sb = const.tile([P, 4], fp32)
        nc.sync.dma_start(bqk_sb[:], bqk.rearrange("(m p) -> p m", p=P))
        bv_sb = const.tile([1, FG], bf16)
        nc.sync.dma_start(bv_sb[:], bv.rearrange("(o f) -> o f", o=1))

        xt_sb = pool("xt").tile([P, KH * T], bf16)
        w_sb = pool("w").tile([P, KH * W3], bf16)
        wp_sb = pool("wp").tile([P, 2 * H], bf16)
        for k in range(KH):
            nc.sync.dma_start(xt_sb[:, k * T : (k + 1) * T], xT[k * P : (k + 1) * P, :])
            nc.sync.dma_start(
                w_sb[:, k * W3 : (k + 1) * W3], wqkv[k * P : (k + 1) * P, :]
            )
        for kk in range(2):
            nc.sync.dma_start(
                wp_sb[:, kk * H : (kk + 1) * H], wp[kk * P : (kk + 1) * P, :]
            )

        qk_sb = pool("qk").tile([P, 4 * T], bf16)   # feat chunks: q0 q1 k0 k1
        # v weight stride padded to 128 columns so PV ldweights gets FWL
        # (4x faster weight load); cols 65..127 are zeros -> junk psum rows
        v_sb = pool("v").tile([P, NI * 4 * P], bf16)
        v4 = v_sb.rearrange("p (t h c) -> p t h c", t=NI, h=4, c=P)
        nc.vector.memset(v_sb[:], 0.0)
        nc.vector.memset(v4[:, :, :, 64:65], 1.0)   # denominator ones columns
        y_sb = pool("y").tile([P, 2 * T], bf16)     # y^T, feat pair chunks x T

        # one psum pool for all phases: tag "s" = 2 slots x 2 banks,
        # tag "pvd" = 4 slots x 1 bank (16KB/partition total)
        ps_pool = ctx.enter_context(tc.tile_pool(name="ps", bufs=2, space="PSUM"))
        epool = pool("e", bufs=3)
        nrm = pool("nrm", bufs=2)
        pvs_pool = pool("pvs", bufs=4)
        outp = pool("outp", bufs=3)

        # ---- Phase A: projections (pair-0 k/q first so attention starts early)
        for m in (2, 0, 3, 1):
            for j in range(NJ):
                ps = ps_pool.tile([P, 2 * NQ], fp32, tag="s", name=f"qk{m}_{j}")
                for k in range(KH):
                    nc.tensor.matmul(
                        ps[:, 0:NQ],
                        w_sb[:, k * W3 + m * P : k * W3 + (m + 1) * P],
                        xt_sb[:, k * T + j * NQ : k * T + (j + 1) * NQ],
                        start=(k == 0),
                        stop=(k == KH - 1),
                    )
                nc.scalar.activation(  # copy+bias on ACT (idle during phase A)
                    qk_sb[:, m * T + j * NQ : m * T + (j + 1) * NQ],
                    ps[:, 0:NQ],
                    Ident,
                    bias=bqk_sb[:, m : m + 1],
                )
        for t in range(NI):  # v in natural layout, strided into [64|1] slots
            ps = ps_pool.tile([P, NQ], fp32, tag="pvd", bufs=4, name=f"v{t}")
            for k in range(KH):
                nc.tensor.matmul(
                    ps[:, 0:FG],
                    xt_sb[:, k * T + t * P : k * T + (t + 1) * P],
                    w_sb[:, k * W3 + 2 * FG : (k + 1) * W3],
                    start=(k == 0),
                    stop=False,
                )
            nc.tensor.matmul(  # += ones.T @ bv  (bias broadcast over rows)
                ps[:, 0:FG],
                ones[0:1, :],
                bv_sb[0:1, :],
                start=False,
                stop=True,
            )
            nc.vector.tensor_copy(
                v4[:, t, :, 0:64],
                ps[:, 0:FG].rearrange("p (h c) -> p h c", h=4, c=64),
            )

        # ---- Phase B: attention ----
        for j in range(NJ):
            pvd = [
                ps_pool.tile([P, NQ], fp32, tag="pvd", bufs=4, name=f"pvd{j}_{h}")
                for h in range(4)
            ]
            for i in range(NI):
                for p in range(2):  # head pairs (2p, 2p+1)
                    sp = ps_pool.tile([P, 2 * NQ], fp32, tag="s", name=f"s{j}_{i}_{p}")
                    for hh in range(2):
                        bp = 64 * hh
                        # S^T chunk: K=64, row-packed -> runs concurrently
                        nc.tensor.matmul(
                            sp[:, hh * NQ : (hh + 1) * NQ],
                            qk_sb[
                                bp : bp + 64,
                                (2 + p) * T + i * P : (2 + p) * T + (i + 1) * P,
                            ],
                            qk_sb[
                                bp : bp + 64, p * T + j * NQ : p * T + (j + 1) * NQ
                            ],
                            start=True,
                            stop=True,
                            tile_position=(bp, 0),
                        )
                    e = epool.tile([P, 2 * NQ], bf16, tag="e")
                    nc.scalar.activation(e[:], sp[:], Exp, bias=bias0[:, 0:1])
                    for hh in range(2):
                        h = 2 * p + hh
                        nc.tensor.matmul(  # y^T rows 0..63, denominator row 64
                            pvd[h][:],
                            v4[:, i, h, :],
                            e[:, hh * NQ : (hh + 1) * NQ],
                            start=(i == 0),
                            stop=(i == NI - 1),
                        )
            # normalize: stage to SBUF (frees pvd banks), batched reciprocal,
            # selector broadcast, all-SBUF multiplies
            pvs = [
                pvs_pool.tile([VC, NQ], fp32, tag="pvs", name=f"pvs{j}_{h}")
                for h in range(4)
            ]
            for h in range(4):
                nc.vector.tensor_copy(pvs[h][:], pvd[h][0:VC, :])
            dg = nrm.tile([P, NQ], fp32, tag="dg")
            nc.vector.memset(dg[:], 1.0)  # keep untouched rows finite
            for h in range(4):
                nc.vector.tensor_copy(dg[32 * h : 32 * h + 1, :], pvs[h][64:65, :])
            rcp = nrm.tile([P, NQ], fp32, tag="rcp")
            nc.vector.reciprocal(rcp[:], dg[:])
            for h in range(4):
                p, hh = divmod(h, 2)
                bc = ps_pool.tile([64, NQ], fp32, tag="pvd", bufs=4, name=f"bc{j}_{h}")
                nc.tensor.matmul(
                    bc[:], sel[:, h * 64 : (h + 1) * 64], rcp[:], start=True, stop=True
                )
                bc_sb = nrm.tile([64, NQ], fp32, tag="bcs")
                nc.scalar.activation(bc_sb[:], bc[:], Copy)
                nc.vector.tensor_mul(  # both SB inputs at base partition 0
                    y_sb[
                        64 * hh : 64 * hh + 64,
                        p * T + j * NQ : p * T + (j + 1) * NQ,
                    ],
                    pvs[h][0:64, :],
                    bc_sb[:],
                )

        # ---- Phase C: c_proj partial (row-parallel) ----
        for mq in range(NI):
            for n in range(2):
                ps = ps_pool.tile([P, NQ], fp32, tag="pvd", bufs=4, name=f"c{mq}_{n}")
                for kk in range(2):
                    nc.tensor.matmul(
                        ps[:],
                        y_sb[:, kk * T + mq * P : kk * T + (mq + 1) * P],
                        wp_sb[:, kk * H + n * NQ : kk * H + (n + 1) * NQ],
                        start=(kk == 0),
                        stop=(kk == 1),
                    )
                ot = outp.tile([P, NQ], fp32, tag="o")
                nc.scalar.activation(ot[:], ps[:], Copy)  # ACT idle in phase C
                nc.sync.dma_start(
                    out[mq * P : (mq + 1) * P, n * NQ : (n + 1) * NQ], ot[:]
                )


def _get_nc():
    if "nc" not in _CACHE:
        _CACHE["nc"] = _build()
    return _CACHE["nc"]


def _make_in_maps(x, W_attn, b_attn, W_proj):
    import ml_dtypes

    bf = ml_dtypes.bfloat16
    x = np.asarray(x, np.float32)
    W_attn = np.asarray(W_attn, np.float32)
    b_attn = np.asarray(b_attn, np.float32)
    W_proj = np.asarray(W_proj, np.float32)
    scale = 1.0 / np.sqrt(np.float32(HD))
    in_maps = []
    for c in range(NCORES):
        b, g = divmod(c, 4)
        sl = slice(FG * g, FG * (g + 1))
        wq = W_attn[:, sl] * scale
        wk = W_attn[:, H:][:, sl]
        wv = W_attn[:, 2 * H :][:, sl]
        in_maps.append(
            {
                "xT": np.ascontiguousarray(x[b].T).astype(bf),
                "wqkv": np.ascontiguousarray(
                    np.concatenate([wq, wk, wv], axis=1)
                ).astype(bf),
                "bqk": np.concatenate(
                    [b_attn[sl] * scale, b_attn[H:][sl]]
                ).astype(np.float32),
                "bv": np.ascontiguousarray(b_attn[2 * H :][sl]).astype(bf),
                "wp": np.ascontiguousarray(W_proj[sl, :]).astype(bf),
            }
        )
    return in_maps


def _gather(results, b_proj):
    b_proj = np.asarray(b_proj, np.float32)
    y = np.empty((B, T, H), np.float32)
    for b in range(B):
        acc = results[4 * b]["out"].astype(np.float32)
        for g in range(1, 4):
            acc = acc + results[4 * b + g]["out"]
        y[b] = acc + b_proj[None, :]
    return y


def run(x, W_attn, b_attn, W_proj, b_proj, trace=False):
    from concourse.bass_utils import run_bass_kernel_spmd

    nc = _get_nc()
    in_maps = _make_in_maps(x, W_attn, b_attn, W_proj)
    res = run_bass_kernel_spmd(nc, in_maps, list(range(NCORES)), trace=trace)
    return _gather(res.results, b_proj), res


def kernel(x, W_attn, b_attn, W_proj, b_proj):
    y, _ = run(x, W_attn, b_attn, W_proj, b_proj, trace=False)
    return y


# revision 16
# speedup vs baseline: 1.0262x; 1.0115x over previous
"""Causal self-attention (non-masked softmax path) for TRN2, 8 NeuronCores.

Sharding: 2-way data parallel over batch x 4-way tensor parallel over heads.
Core c handles batch b = c // 4, head group g = c % 4 (heads 4g..4g+3).
Each core computes its QKV projection slice, full attention for its 4 heads,
and the row-parallel c_proj partial; the host sums the 4 partials per batch
(the all-reduce of row-parallel tensor parallelism) and adds b_proj.

All matmuls take bf16 inputs (fp32 matmul on TRN2 runs as 2 passes at 2
bytes/cycle = 4x slower) and accumulate in fp32 PSUM. Measured end-to-end
error vs the fp32 reference is ~6e-3 scale-relative.

Per-core dataflow:
  xT [1024, 2048] bf16 (host pre-transposed; matmuls contract over partitions)
  qkT = Wqk.T @ x.T [512, 2048] bf16, q pre-scaled by 1/sqrt(hd) via W on host
  v   = x @ Wv [2048, 4 x (64|1)] bf16 -- augmented with a ones column per
        head so the PV matmul (lhsT [128, 65]) also produces the softmax
        denominator in psum row 64.
  S^T chunks [128 k, 512 q] per head, K=64 row-packed 2 heads per PE pass
  (row-group packing runs concurrently; col-group packing does not).
  exp on ACT over pair-fused [128, 1024] psum -> bf16 E tiles.
  Normalize y^T after PV: stage pvd to SBUF (frees psum banks fast), batched
  reciprocal of the 4 denominator rows, selector-matmul broadcast, DVE mul.
  c_proj contracts the 256 group features; partial out stays fp32.

A single PSUM pool with tags "s" (2 x 2 banks) and "pvd" (4 x 1 bank) is
shared by all phases so projection, attention, and c_proj pipeline into
each other instead of serializing on pool-scope boundaries. PSUM->SBUF
copies that would collide with DVE work run on the otherwise-idle ACT
engine (activation Copy/Identity).
"""

import numpy as np

B, T, H, NH, HD = 2, 2048, 1024, 16, 64
P = 128
FG = 256          # features per head group (4 heads x 64)
VC = 65           # v columns per head incl. the ones column
NQ = 512          # Tq chunk (psum free dim)
NJ = T // NQ      # 4
NI = T // P       # 16 key chunks
KH = H // P       # 8 hidden chunks
NCORES = 8

_CACHE = {}


def _build():
    import concourse.bacc as bacc
    import concourse.mybir as mybir
    import concourse.tile as tile

    fp32 = mybir.dt.float32
    bf16 = mybir.dt.bfloat16

    nc = bacc.Bacc("TRN2", debug=False)
    xT = nc.dram_tensor("xT", [H, T], bf16, kind="ExternalInput").ap()
    wqkv = nc.dram_tensor("wqkv", [H, 3 * FG], bf16, kind="ExternalInput").ap()
    bqk = nc.dram_tensor("bqk", [2 * FG], fp32, kind="ExternalInput").ap()
    bv = nc.dram_tensor("bv", [FG], bf16, kind="ExternalInput").ap()
    wp = nc.dram_tensor("wp", [FG, H], bf16, kind="ExternalInput").ap()
    out = nc.dram_tensor("out", [T, H], fp32, kind="ExternalOutput").ap()

    with tile.TileContext(nc) as tc:
        _emit(nc, tc, mybir, xT, wqkv, bqk, bv, wp, out)
    nc.compile()
    return nc


def _emit(nc, tc, mybir, xT, wqkv, bqk, bv, wp, out):
    from contextlib import ExitStack

    fp32 = mybir.dt.float32
    bf16 = mybir.dt.bfloat16
    Exp = mybir.ActivationFunctionType.Exp
    Ident = mybir.ActivationFunctionType.Identity
    Copy = mybir.ActivationFunctionType.Copy

    W3 = 3 * FG  # 768, wqkv row width

    with ExitStack() as ctx:
        pool = lambda name, bufs=1, space="SBUF": ctx.enter_context(
            tc.tile_pool(name=name, bufs=bufs, space=space)
        )

        const = pool("const")
        ones = const.tile([1, P], bf16)
        nc.vector.memset(ones[:], 1.0)
        bias0 = const.tile([P, 1], fp32)
        nc.vector.memset(bias0[:], 0.0)
        # selector weights for the denominator broadcast: sel[k, 64h+m]
        # = 1 iff k == 32h; sel_h.T @ rcp replicates head h's denominator
        # row (32h of rcp) across 64 output partitions starting at base 0.
        sel = const.tile([P, 4 * 64], bf16)
        nc.vector.memset(sel[:], 0.0)
        for h in range(4):
            nc.vector.memset(sel[32 * h : 32 * h + 1, h * 64 : (h + 1) * 64], 1.0)
        bqk_sb = const.tile([P, 4], fp32)
        nc.sync.dma_start(bqk_sb[:], bqk.rearrange("(m p) -> p m", p=P))
        bv_sb = const.tile([1, FG], bf16)
        nc.sync.dma_start(bv_sb[:], bv.rearrange("(o f) -> o f", o=1))

        xt_sb = pool("xt").tile([P, KH * T], bf16)
        w_sb = pool("w").tile([P, KH * W3], bf16)
        wp_sb = pool("wp").tile([P, 2 * H], bf16)
        for k in range(KH):
            nc.sync.dma_start(xt_sb[:, k * T : (k + 1) * T], xT[k * P : (k + 1) * P, :])
            nc.sync.dma_start(
                w_sb[:, k * W3 : (k + 1) * W3], wqkv[k * P : (k + 1) * P, :]
            )
        for kk in range(2):
            nc.sync.dma_start(
                wp_sb[:, kk * H : (kk + 1) * H], wp[kk * P : (kk + 1) * P, :]
            )

        qk_sb = pool("qk").tile([P, 4 * T], bf16)   # feat chunks: q0 q1 k0 k1
        # v weight stride padded to 128 columns so PV ldweights gets FWL
        # (4x faster weight load); cols 65..127 are zeros -> junk psum rows
        v_sb = pool("v").tile([P, NI * 4 * P], bf16)
        v4 = v_sb.rearrange("p (t h c) -> p t h c", t=NI, h=4, c=P)
        nc.vector.memset(v_sb[:], 0.0)
        nc.vector.memset(v4[:, :, :, 64:65], 1.0)   # denominator ones columns
        y_sb = pool("y").tile([P, 2 * T], bf16)     # y^T, feat pair chunks x T

        # one psum pool for all phases: tag "s" = 2 slots x 2 banks,
        # tag "pvd" = 4 slots x 1 bank (16KB/partition total)
        ps_pool = ctx.enter_context(tc.tile_pool(name="ps", bufs=2, space="PSUM"))
        epool = pool("e", bufs=3)
        nrm = pool("nrm", bufs=2)
        pvs_pool = pool("pvs", bufs=4)
        outp = pool("outp", bufs=3)

        # ---- Phase A: projections (pair-0 k/q first so attention starts early)
        for m in (2, 0, 3, 1):
            for j in range(NJ):
                ps = ps_pool.tile([P, 2 * NQ], fp32, tag="s", name=f"qk{m}_{j}")
                for k in range(KH):
                    nc.tensor.matmul(
                        ps[:, 0:NQ],
                        w_sb[:, k * W3 + m * P : k * W3 + (m + 1) * P],
                        xt_sb[:, k * T + j * NQ : k * T + (j + 1) * NQ],
                        start=(k == 0),
                        stop=(k == KH - 1),
                    )
                nc.scalar.activation(  # copy+bias on ACT (idle during phase A)
                    qk_sb[:, m * T + j * NQ : m * T + (j + 1) * NQ],
                    ps[:, 0:NQ],
                    Ident,
                    bias=bqk_sb[:, m : m + 1],
                )
        for t in range(NI):  # v in natural layout, strided into [64|1] slots
            ps = ps_pool.tile([P, NQ], fp32, tag="pvd", bufs=4, name=f"v{t}")
            for k in range(KH):
                nc.tensor.matmul(
                    ps[:, 0:FG],
                    xt_sb[:, k * T + t * P : k * T + (t + 1) * P],
                    w_sb[:, k * W3 + 2 * FG : (k + 1) * W3],
                    start=(k == 0),
                    stop=False,
                )
            nc.tensor.matmul(  # += ones.T @ bv  (bias broadcast over rows)
                ps[:, 0:FG],
                ones[0:1, :],
                bv_sb[0:1, :],
                start=False,
                stop=True,
            )
            nc.vector.tensor_copy(
                v4[:, t, :, 0:64],
                ps[:, 0:FG].rearrange("p (h c) -> p h c", h=4, c=64),
            )

        # ---- Phase B: attention ----
        for j in range(NJ):
            pvd = [
                ps_pool.tile([P, NQ], fp32, tag="pvd", bufs=4, name=f"pvd{j}_{h}")
                for h in range(4)
            ]
            for i in range(NI):
                for p in range(2):  # head pairs (2p, 2p+1)
                    sp = ps_pool.tile([P, 2 * NQ], fp32, tag="s", name=f"s{j}_{i}_{p}")
                    for hh in range(2):
                        bp = 64 * hh
                        # S^T chunk: K=64, row-packed -> runs concurrently
                        nc.tensor.matmul(
                            sp[:, hh * NQ : (hh + 1) * NQ],
                            qk_sb[
                                bp : bp + 64,
                                (2 + p) * T + i * P : (2 + p) * T + (i + 1) * P,
                            ],
                            qk_sb[
                                bp : bp + 64, p * T + j * NQ : p * T + (j + 1) * NQ
                            ],
                            start=True,
                            stop=True,
                            tile_position=(bp, 0),
                        )
                    e = epool.tile([P, 2 * NQ], bf16, tag="e")
                    nc.scalar.activation(e[:], sp[:], Exp, bias=bias0[:, 0:1])
                    for hh in range(2):
                        h = 2 * p + hh
                        nc.tensor.matmul(  # y^T rows 0..63, denominator row 64
                            pvd[h][:],
                            v4[:, i, h, :],
                            e[:, hh * NQ : (hh + 1) * NQ],
                            start=(i == 0),
                            stop=(i == NI - 1),
                        )
            # normalize: stage to SBUF (frees pvd banks), batched reciprocal,
            # selector broadcast, all-SBUF multiplies
            pvs = [
                pvs_pool.tile([VC, NQ], fp32, tag="pvs", name=f"pvs{j}_{h}")
                for h in range(4)
            ]
            for h in range(4):
                nc.vector.tensor_copy(pvs[h][:], pvd[h][0:VC, :])
            dg = nrm.tile([P, NQ], fp32, tag="dg")
            nc.vector.memset(dg[:], 1.0)  # keep untouched rows finite
            for h in range(4):
                nc.vector.tensor_copy(dg[32 * h : 32 * h + 1, :], pvs[h][64:65, :])
            rcp = nrm.tile([P, NQ], bf16, tag="rcp")
            with nc.allow_low_precision(reason="softmax denom broadcast in bf16"):
                nc.vector.reciprocal(rcp[:], dg[:])
            for h in range(4):
                p, hh = divmod(h, 2)
                bc = ps_pool.tile([64, NQ], fp32, tag="pvd", bufs=4, name=f"bc{j}_{h}")
                nc.tensor.matmul(
                    bc[:], sel[:, h * 64 : (h + 1) * 64], rcp[:], start=True, stop=True
                )
                bc_sb = nrm.tile([64, NQ], fp32, tag="bcs")
                nc.scalar.activation(bc_sb[:], bc[:], Copy)
                nc.vector.tensor_mul(  # both SB inputs at base partition 0
                    y_sb[
                        64 * hh : 64 * hh + 64,
                        p * T + j * NQ : p * T + (j + 1) * NQ,
                    ],
                    pvs[h][0:64, :],
                    bc_sb[:],
                )

        # ---- Phase C: c_proj partial (row-parallel) ----
        for mq in range(NI):
            pcs = [
                ps_pool.tile([P, NQ], fp32, tag="pvd", bufs=4, name=f"c{mq}_{n}")
                for n in range(2)
            ]
            for kk in range(2):  # kk outer: each y lhsT loads once for both n
                for n in range(2):
                    nc.tensor.matmul(
                        pcs[n][:],
                        y_sb[:, kk * T + mq * P : kk * T + (mq + 1) * P],
                        wp_sb[:, kk * H + n * NQ : kk * H + (n + 1) * NQ],
                        start=(kk == 0),
                        stop=(kk == 1),
                    )
            for n in range(2):
                ot = outp.tile([P, NQ], fp32, tag="o")
                nc.scalar.activation(ot[:], pcs[n][:], Copy)  # ACT idle in phase C
                nc.sync.dma_start(
                    out[mq * P : (mq + 1) * P, n * NQ : (n + 1) * NQ], ot[:]
                )


def _get_nc():
    if "nc" not in _CACHE:
        _CACHE["nc"] = _build()
    return _CACHE["nc"]


def _make_in_maps(x, W_attn, b_attn, W_proj):
    import ml_dtypes

    bf = ml_dtypes.bfloat16
    x = np.asarray(x, np.float32)
    W_attn = np.asarray(W_attn, np.float32)
    b_attn = np.asarray(b_attn, np.float32)
    W_proj = np.asarray(W_proj, np.float32)
    scale = 1.0 / np.sqrt(np.float32(HD))
    in_maps = []
    for c in range(NCORES):
        b, g = divmod(c, 4)
        sl = slice(FG * g, FG * (g + 1))
        wq = W_attn[:, sl] * scale
        wk = W_attn[:, H:][:, sl]
        wv = W_attn[:, 2 * H :][:, sl]
        in_maps.append(
            {
                "xT": np.ascontiguousarray(x[b].T).astype(bf),
                "wqkv": np.ascontiguousarray(
                    np.concatenate([wq, wk, wv], axis=1)
                ).astype(bf),
                "bqk": np.concatenate(
                    [b_attn[sl] * scale, b_attn[H:][sl]]
                ).astype(np.float32),
                "bv": np.ascontiguousarray(b_attn[2 * H :][sl]).astype(bf),
                "wp": np.ascontiguousarray(W_proj[sl, :]).astype(bf),
            }
        )
    return in_maps


def _gather(results, b_proj):
    b_proj = np.asarray(b_proj, np.float32)
    y = np.empty((B, T, H), np.float32)
    for b in range(B):
        acc = results[4 * b]["out"].astype(np.float32)
        for g in range(1, 4):
            acc = acc + results[4 * b + g]["out"]
        y[b] = acc + b_proj[None, :]
    return y


def run(x, W_attn, b_attn, W_proj, b_proj, trace=False):
    from concourse.bass_utils import run_bass_kernel_spmd

    nc = _get_nc()
    in_maps = _make_in_maps(x, W_attn, b_attn, W_proj)
    res = run_bass_kernel_spmd(nc, in_maps, list(range(NCORES)), trace=trace)
    return _gather(res.results, b_proj), res


def kernel(x, W_attn, b_attn, W_proj, b_proj):
    y, _ = run(x, W_attn, b_attn, W_proj, b_proj, trace=False)
    return y
